# revision 3
# baseline (speedup 1.0000x reference)
"""Multi-head causal self-attention on 8 Trainium2 NeuronCores.

Sharding: heads are split 2-per-core (tensor parallel); every core computes
QKV projections for its 2 heads over the full batch, runs attention, then an
8-core AllToAll redistributes the per-head outputs to sequence-sharded form
for the row-parallel output projection. Matmuls run in float32r (TF32-like).

Reference semantics (torch nn.Linear convention, y = x @ W.T):
  Q = x @ Wq.T ; K = x @ Wk.T ; V = x @ Wv.T           (split into 16 heads)
  scores = Q K^T / sqrt(64), causal-masked, softmax
  out = (softmax(scores) @ V, concat heads) @ Wo.T + bo
"""

import os
import sys

sys.path.insert(0, "/opt/trn_rl_repo")

import numpy as np

import concourse.bass as bass  # noqa: E402
import concourse.mybir as mybir  # noqa: E402
from concourse import bacc  # noqa: E402
from concourse.bass_utils import run_bass_kernel_spmd  # noqa: E402
from concourse.masks import make_identity  # noqa: E402
from concourse.tile import TileContext  # noqa: E402

B = 2
S = 2048
D = 1024
H = 16
DK = 64
N_CORES = 8
HPC = H // N_CORES          # heads per core = 2
EL = HPC * DK               # local embedding slice = 128
P = 128                     # partitions
SBLK = 512                  # q-block (free dim of score matmuls)
NQ = S // SBLK              # q-blocks per batch = 4
NKT = S // P                # k-tiles per batch = 16
ND = D // P                 # d-tiles = 8
BS = B * S                  # flattened (batch, seq) = 4096
F32 = mybir.dt.float32
F32R = mybir.dt.float32r


def _classify_mask(mask: np.ndarray):
    """Classify each (q-block, k-tile) block of the [S, S] mask.

    Returns (blocks, patterns):
      blocks[j] = list of (t, pat_idx or None) k-tiles with any valid entry
      patterns  = float32 [n_pat, P, SBLK] multiplicative masks in [k, q]
                  layout for partially-valid blocks (deduplicated).
    """
    mask = np.asarray(mask).astype(bool)
    patterns = []
    pat_index = {}
    blocks = []
    for j in range(NQ):
        row = []
        sub_q = mask[j * SBLK:(j + 1) * SBLK]
        for t in range(NKT):
            sub = sub_q[:, t * P:(t + 1) * P]
            if not sub.any():
                continue
            if sub.all():
                row.append((t, None))
                continue
            patT = np.ascontiguousarray(sub.T).astype(np.float32)  # [k, q]
            key = patT.tobytes()
            if key not in pat_index:
                pat_index[key] = len(patterns)
                patterns.append(patT)
            row.append((t, pat_index[key]))
        assert row, f"q-block {j} has no valid keys; unsupported mask"
        blocks.append(row)
    if not patterns:
        patterns.append(np.ones((P, SBLK), np.float32))
    return blocks, np.stack(patterns)


def _build_program(blocks, n_pat):
    nc = bacc.Bacc("TRN2", target_bir_lowering=False, debug=False,
                   num_devices=N_CORES)

    # ---- I/O ----------------------------------------------------------
    # xT: [D, B*S] (x transposed, batches concatenated along columns)
    xT = nc.declare_dram_parameter("xT", [D, BS], F32R, isOutput=False)
    wqT = nc.declare_dram_parameter("wqT", [D, EL], F32R, isOutput=False)
    wkT = nc.declare_dram_parameter("wkT", [D, EL], F32R, isOutput=False)
    wvT = nc.declare_dram_parameter("wvT", [D, EL], F32R, isOutput=False)
    woT = nc.declare_dram_parameter("woT", [D, D], F32R, isOutput=False)
    bo = nc.declare_dram_parameter("bo", [1, D], F32, isOutput=False)
    mpat = nc.declare_dram_parameter("mpat", [n_pat, P, SBLK], F32R,
                                     isOutput=False)
    out = nc.declare_dram_parameter("out", [SBLK, D], F32, isOutput=True)

    # collective bounce buffers (internal DRAM)
    a2a_in = nc.dram_tensor("a2a_in", [N_CORES, P, SBLK], F32R)
    a2a_out = nc.dram_tensor("a2a_out", [N_CORES, P, SBLK], F32R)

    with TileContext(nc) as tc:
        from contextlib import ExitStack
        with ExitStack() as ctx:
            const = ctx.enter_context(tc.tile_pool(name="const", bufs=1))
            persist = ctx.enter_context(tc.tile_pool(name="persist", bufs=1))

            # constants
            ident = const.tile([P, P], F32)
            make_identity(nc, ident)
            bo_sb = const.tile([1, D], F32)
            nc.sync.dma_start(out=bo_sb[:], in_=bo[:])
            bo_bc = const.tile([P, D], F32)
            nc.gpsimd.partition_broadcast(bo_bc[:], bo_sb[:])
            mpat_sb = const.tile([P, n_pat * SBLK], F32R, tag="mpat")
            nc.sync.dma_start(
                out=mpat_sb[:].rearrange("p (n q) -> p n q", n=n_pat),
                in_=mpat[:].rearrange("n p q -> p n q"))

            # weights for the QKV projections: [P, ND*EL], d-tile k at
            # columns [k*EL, (k+1)*EL)
            w_sb = {}
            for name, t in (("wq", wqT), ("wk", wkT), ("wv", wvT)):
                w = const.tile([P, ND * EL], F32R, name=f"w_{name}",
                               tag=f"w_{name}")
                nc.sync.dma_start(
                    out=w[:].rearrange("p (k e) -> p k e", k=ND),
                    in_=t[:].rearrange("(k p) e -> p k e", p=P))
                w_sb[name] = w

            # persistent activations
            qT = persist.tile([P, BS], F32R, tag="qT")   # [EL, B*S]
            kT = persist.tile([P, BS], F32R, tag="kT")
            # V_aug: per k-tile [P, 2*65]; head h at cols [h*65, h*65+64],
            # ones at col h*65+64
            v_aug = [persist.tile([P, HPC * (DK + 1)], F32R,
                                  name=f"vaug{i}", tag=f"vaug{i}")
                     for i in range(B * NKT)]
            stage = [persist.tile([P, SBLK], F32R, name=f"stg{i}",
                                  tag=f"stg{i}")
                     for i in range(N_CORES)]
            woT_sb = persist.tile([P, ND * D], F32R, tag="woT")

            # ---- phase 1: projections -------------------------------
            with ExitStack() as p1:
                xpool = p1.enter_context(tc.tile_pool(name="xT", bufs=ND + 1))
                vt_pool = p1.enter_context(tc.tile_pool(name="vt", bufs=1))
                ps_qk = p1.enter_context(
                    tc.tile_pool(name="ps_qk", bufs=3, space="PSUM"))
                ps_tr = p1.enter_context(
                    tc.tile_pool(name="ps_tr", bufs=2, space="PSUM"))

                HALF = BS // 2
                for half in range(2):
                    xt = []
                    for k in range(ND):
                        t = xpool.tile([P, HALF], F32R, tag="xt")
                        nc.sync.dma_start(
                            out=t[:],
                            in_=xT[k * P:(k + 1) * P,
                                   half * HALF:(half + 1) * HALF])
                        xt.append(t)
                    vt = vt_pool.tile([P, HALF], F32R, tag="vt")
                    for sb in range(HALF // SBLK):
                        ssl = bass.ts(sb, SBLK)
                        for name, dest in (("wq", qT), ("wk", kT),
                                           ("wv", vt)):
                            ps = ps_qk.tile([P, SBLK], F32, tag="ps_qk")
                            for k in range(ND):
                                nc.tensor.matmul(
                                    ps[:], w_sb[name][:, bass.ts(k, EL)],
                                    xt[k][:, ssl],
                                    start=(k == 0), stop=(k == ND - 1))
                            if name == "wv":
                                nc.vector.tensor_copy(vt[:, ssl], ps[:])
                            else:
                                osl = bass.ds(half * HALF + sb * SBLK, SBLK)
                                nc.vector.tensor_copy(dest[:, osl], ps[:])
                    # transpose VT -> V (per [P, P] block), augment ones
                    for i in range(HALF // P):
                        tile_idx = half * (HALF // P) + i
                        ps = ps_tr.tile([P, P], F32, tag="ps_tr")
                        nc.tensor.matmul(ps[:],
                                         vt[:, bass.ts(i, P)].bitcast(F32),
                                         ident[:], is_transpose=True)
                        va = v_aug[tile_idx]
                        nc.vector.tensor_copy(
                            va[:].rearrange("p (h e) -> p h e",
                                            h=HPC)[:, :, 0:DK],
                            ps[:].rearrange("p (h e) -> p h e", h=HPC))
                        nc.vector.memset(
                            va[:].rearrange("p (h e) -> p h e",
                                            h=HPC)[:, :, DK:DK + 1]
                            .bitcast(F32), 1.0)

            # WoT load (needed only in phase 3; issued here so the DMA
            # overlaps the attention phase)
            nc.sync.dma_start(
                out=woT_sb[:].rearrange("p (k e) -> p k e", k=ND),
                in_=woT[:].rearrange("(k p) e -> p k e", p=P))

            # ---- phase 2: attention ---------------------------------
            with ExitStack() as p2:
                probs_pool = p2.enter_context(tc.tile_pool(name="probs",
                                                           bufs=3))
                small = p2.enter_context(tc.tile_pool(name="small", bufs=4))
                ps_sc = p2.enter_context(
                    tc.tile_pool(name="ps_sc", bufs=2, space="PSUM"))
                ps_out = p2.enter_context(
                    tc.tile_pool(name="ps_out", bufs=2, space="PSUM"))

                mpat3 = mpat_sb[:].rearrange("p (n q) -> p n q", n=n_pat)
                for b in range(B):
                    for j in range(NQ):
                        for h in range(HPC):
                            hsl = bass.ds(h * DK, DK)
                            q0 = b * S + j * SBLK
                            tiles = blocks[j]
                            po = ps_out.tile([P, SBLK], F32, tag="ps_out")
                            n_mm = 0
                            for c0 in range(0, len(tiles), 2):
                                pair = tiles[c0:c0 + 2]
                                w = len(pair) * SBLK
                                ps = ps_sc.tile([P, 2 * SBLK], F32,
                                                tag="ps_sc")
                                for i, (t, _pat) in enumerate(pair):
                                    nc.tensor.matmul(
                                        ps[:, bass.ts(i, SBLK)],
                                        kT[hsl, bass.ds(b * S + t * P, P)],
                                        qT[hsl, bass.ds(q0, SBLK)],
                                        start=True, stop=True)
                                pr = probs_pool.tile([P, 2 * SBLK], F32R,
                                                     tag="probs")
                                nc.scalar.activation(
                                    pr[:, 0:w], ps[:, 0:w],
                                    mybir.ActivationFunctionType.Exp)
                                for i, (t, pat) in enumerate(pair):
                                    if pat is not None:
                                        nc.vector.tensor_mul(
                                            pr[:, bass.ts(i, SBLK)],
                                            pr[:, bass.ts(i, SBLK)],
                                            mpat3[:, pat])
                                for i, (t, _pat) in enumerate(pair):
                                    n_mm += 1
                                    nc.tensor.matmul(
                                        po[0:DK + 1, :],
                                        v_aug[b * NKT + t][
                                            :, bass.ds(h * (DK + 1),
                                                       DK + 1)],
                                        pr[:, bass.ts(i, SBLK)],
                                        start=(n_mm == 1),
                                        stop=(n_mm == len(tiles)))
                            recip = small.tile([1, SBLK], F32, tag="recip")
                            nc.vector.reciprocal(recip[:],
                                                 po[DK:DK + 1, :])
                            rb = small.tile([DK, SBLK], F32, tag="rb")
                            nc.gpsimd.partition_broadcast(rb[:], recip[:])
                            nc.vector.tensor_mul(
                                stage[b * NQ + j][h * DK:(h + 1) * DK, :],
                                po[0:DK, :], rb[:])
                        nc.sync.dma_start(out=a2a_in[b * NQ + j],
                                          in_=stage[b * NQ + j][:])

            # ---- phase 3: AllToAll + output projection --------------
            nc.gpsimd.collective_compute(
                "AllToAll", mybir.AluOpType.bypass,
                replica_groups=[list(range(N_CORES))],
                ins=[a2a_in[:]], outs=[a2a_out[:]])

            with ExitStack() as p3:
                cat_pool = p3.enter_context(tc.tile_pool(name="cat", bufs=1))
                osb_pool = p3.enter_context(tc.tile_pool(name="osb", bufs=3))
                ps_f = p3.enter_context(
                    tc.tile_pool(name="ps_f", bufs=3, space="PSUM"))

                cat = []
                for i in range(N_CORES):
                    t = cat_pool.tile([P, SBLK], F32R, name=f"cat{i}",
                                      tag=f"cat{i}")
                    nc.sync.dma_start(out=t[:], in_=a2a_out[i])
                    cat.append(t)
                for st in range(SBLK // P):
                    for eb in range(D // SBLK):
                        ps = ps_f.tile([P, SBLK], F32, tag="ps_f")
                        for k in range(ND):
                            nc.tensor.matmul(
                                ps[:], cat[k][:, bass.ts(st, P)],
                                woT_sb[:, bass.ds(k * D + eb * SBLK, SBLK)],
                                start=(k == 0), stop=(k == ND - 1))
                        ot = osb_pool.tile([P, SBLK], F32, tag="osb")
                        nc.vector.tensor_add(ot[:], ps[:],
                                             bo_bc[:, bass.ts(eb, SBLK)])
                        nc.sync.dma_start(
                            out=out[st * P:(st + 1) * P,
                                    eb * SBLK:(eb + 1) * SBLK],
                            in_=ot[:])

    nc.compile()
    return nc


def _prepare_inputs(x, Wq, Wk, Wv, Wo, bo, patterns):
    x = np.asarray(x, np.float32)
    xT = np.ascontiguousarray(
        np.concatenate([x[b].T for b in range(B)], axis=1))
    woT = np.ascontiguousarray(np.asarray(Wo, np.float32).T)
    bo2 = np.asarray(bo, np.float32).reshape(1, D)
    scale = np.float32(1.0 / np.sqrt(DK))
    in_maps = []
    for c in range(N_CORES):
        cols = slice(c * EL, (c + 1) * EL)
        in_maps.append({
            "xT": xT,
            "wqT": np.ascontiguousarray(np.asarray(Wq, np.float32).T[:, cols]
                                        * scale),
            "wkT": np.ascontiguousarray(np.asarray(Wk, np.float32).T[:, cols]),
            "wvT": np.ascontiguousarray(np.asarray(Wv, np.float32).T[:, cols]),
            "woT": woT,
            "bo": bo2,
            "mpat": patterns,
        })
    return in_maps


def _run(inputs, trace=False):
    blocks, patterns = _classify_mask(inputs["mask"])
    nc = _build_program(blocks, patterns.shape[0])
    in_maps = _prepare_inputs(inputs["x"], inputs["Wq"], inputs["Wk"],
                              inputs["Wv"], inputs["Wo"], inputs["bo"],
                              patterns)
    res = run_bass_kernel_spmd(nc, in_maps, list(range(N_CORES)),
                               trace=trace)
    full = np.empty((B, S, D), np.float32)
    for c in range(N_CORES):
        b, j = divmod(c, NQ)
        full[b, j * SBLK:(j + 1) * SBLK, :] = res.results[c]["out"]
    return full, res


def kernel(**inputs) -> np.ndarray:
    out, _ = _run(inputs, trace=False)
    return out


# revision 5
# speedup vs baseline: 1.1767x; 1.1767x over previous
"""Multi-head causal self-attention on 8 Trainium2 NeuronCores.

Sharding: heads are split 2-per-core (tensor parallel); every core computes
QKV projections for its 2 heads over the full batch, runs attention, then an
8-core AllToAll redistributes the per-head outputs to sequence-sharded form
for the row-parallel output projection. Matmuls run in float32r (TF32-like).

Reference semantics (torch nn.Linear convention, y = x @ W.T):
  Q = x @ Wq.T ; K = x @ Wk.T ; V = x @ Wv.T           (split into 16 heads)
  scores = Q K^T / sqrt(64), causal-masked, softmax
  out = (softmax(scores) @ V, concat heads) @ Wo.T + bo
"""

import os
import sys

sys.path.insert(0, "/opt/trn_rl_repo")

import numpy as np

import concourse.bass as bass  # noqa: E402
import concourse.mybir as mybir  # noqa: E402
from concourse import bacc  # noqa: E402
from concourse.bass_utils import run_bass_kernel_spmd  # noqa: E402
from concourse.masks import make_identity  # noqa: E402
from concourse.tile import TileContext  # noqa: E402

B = 2
S = 2048
D = 1024
H = 16
DK = 64
N_CORES = 8
HPC = H // N_CORES          # heads per core = 2
EL = HPC * DK               # local embedding slice = 128
P = 128                     # partitions
SBLK = 512                  # q-block (free dim of score matmuls)
NQ = S // SBLK              # q-blocks per batch = 4
NKT = S // P                # k-tiles per batch = 16
ND = D // P                 # d-tiles = 8
BS = B * S                  # flattened (batch, seq) = 4096
F32 = mybir.dt.float32
F32R = mybir.dt.float32r


def _classify_mask(mask: np.ndarray):
    """Classify each (q-block, k-tile) block of the [S, S] mask.

    Returns (blocks, patterns):
      blocks[j] = list of (t, pat_idx or None) k-tiles with any valid entry
      patterns  = float32 [n_pat, P, SBLK] multiplicative masks in [k, q]
                  layout for partially-valid blocks (deduplicated).
    """
    mask = np.asarray(mask).astype(bool)
    patterns = []
    pat_index = {}
    blocks = []
    for j in range(NQ):
        row = []
        sub_q = mask[j * SBLK:(j + 1) * SBLK]
        for t in range(NKT):
            sub = sub_q[:, t * P:(t + 1) * P]
            if not sub.any():
                continue
            if sub.all():
                row.append((t, None))
                continue
            patT = np.ascontiguousarray(sub.T).astype(np.float32)  # [k, q]
            key = patT.tobytes()
            if key not in pat_index:
                pat_index[key] = len(patterns)
                patterns.append(patT)
            row.append((t, pat_index[key]))
        assert row, f"q-block {j} has no valid keys; unsupported mask"
        blocks.append(row)
    if not patterns:
        patterns.append(np.ones((P, SBLK), np.float32))
    return blocks, np.stack(patterns)


def _build_program(blocks, n_pat):
    nc = bacc.Bacc("TRN2", target_bir_lowering=False, debug=False,
                   num_devices=N_CORES)

    # ---- I/O ----------------------------------------------------------
    # xT: [D, B*S] (x transposed, batches concatenated along columns)
    xT = nc.declare_dram_parameter("xT", [D, BS], F32R, isOutput=False)
    wqT = nc.declare_dram_parameter("wqT", [P, ND * EL], F32R,
                                    isOutput=False)
    wkT = nc.declare_dram_parameter("wkT", [P, ND * EL], F32R,
                                    isOutput=False)
    wvT = nc.declare_dram_parameter("wvT", [P, ND * EL], F32R,
                                    isOutput=False)
    woT = nc.declare_dram_parameter("woT", [P, ND * D], F32R,
                                    isOutput=False)
    bo = nc.declare_dram_parameter("bo", [1, D], F32, isOutput=False)
    mpat = nc.declare_dram_parameter("mpat", [P, n_pat * SBLK], F32R,
                                     isOutput=False)
    out = nc.declare_dram_parameter("out", [SBLK, D], F32, isOutput=True)

    # collective bounce buffers (internal DRAM)
    a2a_in = nc.dram_tensor("a2a_in", [N_CORES, P, SBLK], F32R)
    a2a_out = nc.dram_tensor("a2a_out", [N_CORES, P, SBLK], F32R)

    with TileContext(nc) as tc:
        from contextlib import ExitStack
        with ExitStack() as ctx:
            const = ctx.enter_context(tc.tile_pool(name="const", bufs=1))
            persist = ctx.enter_context(tc.tile_pool(name="persist", bufs=1))

            # constants
            ident = const.tile([P, P], F32)
            make_identity(nc, ident)
            bo_sb = const.tile([1, D], F32)
            nc.sync.dma_start(out=bo_sb[:], in_=bo[:])
            bo_bc = const.tile([P, D], F32)
            nc.gpsimd.partition_broadcast(bo_bc[:], bo_sb[:])
            mpat_sb = const.tile([P, n_pat * SBLK], F32R, tag="mpat")
            nc.sync.dma_start(out=mpat_sb[:], in_=mpat[:])

            # weights for the QKV projections: [P, ND*EL], d-tile k at
            # columns [k*EL, (k+1)*EL)
            w_sb = {}
            for name, t in (("wq", wqT), ("wk", wkT), ("wv", wvT)):
                w = const.tile([P, ND * EL], F32R, name=f"w_{name}",
                               tag=f"w_{name}")
                nc.sync.dma_start(out=w[:], in_=t[:])
                w_sb[name] = w

            # persistent activations
            qT = persist.tile([P, BS], F32R, tag="qT")   # [EL, B*S]
            kT = persist.tile([P, BS], F32R, tag="kT")
            # V_aug: per k-tile [P, 2*65]; head h at cols [h*65, h*65+64],
            # ones at col h*65+64
            v_aug = [persist.tile([P, HPC * (DK + 1)], F32R,
                                  name=f"vaug{i}", tag=f"vaug{i}")
                     for i in range(B * NKT)]
            stage = [persist.tile([P, SBLK], F32R, name=f"stg{i}",
                                  tag=f"stg{i}")
                     for i in range(N_CORES)]
            woT_sb = persist.tile([P, ND * D], F32R, tag="woT")

            # ---- phase 1: projections -------------------------------
            with ExitStack() as p1:
                xpool = p1.enter_context(tc.tile_pool(name="xT", bufs=ND + 1))
                vt_pool = p1.enter_context(tc.tile_pool(name="vt", bufs=1))
                ps_qk = p1.enter_context(
                    tc.tile_pool(name="ps_qk", bufs=4, space="PSUM"))
                ps_tr = p1.enter_context(
                    tc.tile_pool(name="ps_tr", bufs=2, space="PSUM"))

                HALF = BS // 2
                for half in range(2):
                    xt = []
                    for k in range(ND):
                        t = xpool.tile([P, HALF], F32R, tag="xt")
                        nc.sync.dma_start(
                            out=t[:],
                            in_=xT[k * P:(k + 1) * P,
                                   half * HALF:(half + 1) * HALF])
                        xt.append(t)
                    vt = vt_pool.tile([P, HALF], F32R, tag="vt")
                    nsb = HALF // SBLK
                    for name, dest in (("wq", qT), ("wk", kT),
                                       ("wv", vt)):
                        pss = [ps_qk.tile([P, SBLK], F32, tag="ps_qk",
                                          name=f"ps_{name}_{half}_{sb}")
                               for sb in range(nsb)]
                        for k in range(ND):
                            for sb in range(nsb):
                                nc.tensor.matmul(
                                    pss[sb][:],
                                    w_sb[name][:, bass.ts(k, EL)],
                                    xt[k][:, bass.ts(sb, SBLK)],
                                    start=(k == 0), stop=(k == ND - 1))
                        for sb in range(nsb):
                            if name == "wv":
                                nc.vector.tensor_copy(
                                    vt[:, bass.ts(sb, SBLK)], pss[sb][:])
                            else:
                                osl = bass.ds(half * HALF + sb * SBLK, SBLK)
                                nc.vector.tensor_copy(dest[:, osl],
                                                      pss[sb][:])
                    # transpose VT -> V (per [P, P] block), augment ones
                    for i in range(HALF // P):
                        tile_idx = half * (HALF // P) + i
                        ps = ps_tr.tile([P, P], F32, tag="ps_tr")
                        nc.tensor.matmul(ps[:],
                                         vt[:, bass.ts(i, P)].bitcast(F32),
                                         ident[:], is_transpose=True)
                        va = v_aug[tile_idx]
                        nc.vector.tensor_copy(
                            va[:].rearrange("p (h e) -> p h e",
                                            h=HPC)[:, :, 0:DK],
                            ps[:].rearrange("p (h e) -> p h e", h=HPC))
                        nc.vector.memset(
                            va[:].rearrange("p (h e) -> p h e",
                                            h=HPC)[:, :, DK:DK + 1]
                            .bitcast(F32), 1.0)

            # WoT load (needed only in phase 3; issued here so the DMA
            # overlaps the attention phase)
            nc.sync.dma_start(out=woT_sb[:], in_=woT[:])

            # ---- phase 2: attention ---------------------------------
            with ExitStack() as p2:
                probs_pool = p2.enter_context(tc.tile_pool(name="probs",
                                                           bufs=5))
                small = p2.enter_context(tc.tile_pool(name="small", bufs=4))
                ps_sc = p2.enter_context(
                    tc.tile_pool(name="ps_sc", bufs=3, space="PSUM"))
                ps_out = p2.enter_context(
                    tc.tile_pool(name="ps_out", bufs=2, space="PSUM"))

                mpat3 = mpat_sb[:].rearrange("p (n q) -> p n q", n=n_pat)
                for b in range(B):
                    for j in range(NQ):
                        for h in range(HPC):
                            hsl = bass.ds(h * DK, DK)
                            q0 = b * S + j * SBLK
                            tiles = blocks[j]
                            po = ps_out.tile([P, SBLK], F32, tag="ps_out")
                            n_mm = 0
                            for c0 in range(0, len(tiles), 2):
                                pair = tiles[c0:c0 + 2]
                                w = len(pair) * SBLK
                                ps = ps_sc.tile([P, 2 * SBLK], F32,
                                                tag="ps_sc")
                                for i, (t, _pat) in enumerate(pair):
                                    nc.tensor.matmul(
                                        ps[:, bass.ts(i, SBLK)],
                                        kT[hsl, bass.ds(b * S + t * P, P)],
                                        qT[hsl, bass.ds(q0, SBLK)],
                                        start=True, stop=True)
                                pr = probs_pool.tile([P, 2 * SBLK], F32R,
                                                     tag="probs")
                                nc.scalar.activation(
                                    pr[:, 0:w], ps[:, 0:w],
                                    mybir.ActivationFunctionType.Exp)
                                for i, (t, pat) in enumerate(pair):
                                    if pat is not None:
                                        nc.vector.tensor_mul(
                                            pr[:, bass.ts(i, SBLK)],
                                            pr[:, bass.ts(i, SBLK)],
                                            mpat3[:, pat])
                                for i, (t, _pat) in enumerate(pair):
                                    n_mm += 1
                                    nc.tensor.matmul(
                                        po[0:DK + 1, :],
                                        v_aug[b * NKT + t][
                                            :, bass.ds(h * (DK + 1),
                                                       DK + 1)],
                                        pr[:, bass.ts(i, SBLK)],
                                        start=(n_mm == 1),
                                        stop=(n_mm == len(tiles)))
                            sumb = small.tile([DK, SBLK], F32,
                                              tag="sumb")
                            nc.vector.tensor_copy(sumb[0:1, :],
                                                  po[DK:DK + 1, :])
                            rb = small.tile([DK, SBLK], F32, tag="rb")
                            nc.gpsimd.partition_broadcast(rb[:],
                                                          sumb[0:1, :])
                            nc.vector.reciprocal_approx_fast(out=rb[:],
                                                             in_=rb[:])
                            nc.vector.tensor_mul(
                                stage[b * NQ + j][h * DK:(h + 1) * DK, :],
                                po[0:DK, :], rb[:])
                        nc.sync.dma_start(out=a2a_in[b * NQ + j],
                                          in_=stage[b * NQ + j][:])

            # ---- phase 3: AllToAll + output projection --------------
            nc.gpsimd.collective_compute(
                "AllToAll", mybir.AluOpType.bypass,
                replica_groups=[list(range(N_CORES))],
                ins=[a2a_in[:]], outs=[a2a_out[:]])

            with ExitStack() as p3:
                cat_pool = p3.enter_context(tc.tile_pool(name="cat", bufs=1))
                osb_pool = p3.enter_context(tc.tile_pool(name="osb", bufs=3))
                ps_f = p3.enter_context(
                    tc.tile_pool(name="ps_f", bufs=3, space="PSUM"))

                cat = []
                for i in range(N_CORES):
                    t = cat_pool.tile([P, SBLK], F32R, name=f"cat{i}",
                                      tag=f"cat{i}")
                    nc.sync.dma_start(out=t[:], in_=a2a_out[i])
                    cat.append(t)
                for st in range(SBLK // P):
                    pss = [ps_f.tile([P, SBLK], F32, tag="ps_f",
                                     name=f"ps_f_{st}_{eb}")
                           for eb in range(D // SBLK)]
                    for k in range(ND):
                        for eb in range(D // SBLK):
                            nc.tensor.matmul(
                                pss[eb][:], cat[k][:, bass.ts(st, P)],
                                woT_sb[:, bass.ds(k * D + eb * SBLK, SBLK)],
                                start=(k == 0), stop=(k == ND - 1))
                    for eb in range(D // SBLK):
                        ot = osb_pool.tile([P, SBLK], F32, tag="osb")
                        nc.vector.tensor_add(ot[:], pss[eb][:],
                                             bo_bc[:, bass.ts(eb, SBLK)])
                        nc.sync.dma_start(
                            out=out[st * P:(st + 1) * P,
                                    eb * SBLK:(eb + 1) * SBLK],
                            in_=ot[:])

    nc.compile()
    return nc


def _sbuf_tiled(wT):
    # [D, E] -> [P, ND*E]: row p holds d-tiles k at columns [k*E, (k+1)*E)
    dd, e = wT.shape
    return np.ascontiguousarray(
        wT.reshape(dd // P, P, e).transpose(1, 0, 2).reshape(P, -1))


def _prepare_inputs(x, Wq, Wk, Wv, Wo, bo, patterns):
    x = np.asarray(x, np.float32)
    xT = np.ascontiguousarray(
        np.concatenate([x[b].T for b in range(B)], axis=1))
    woT = _sbuf_tiled(np.ascontiguousarray(np.asarray(Wo, np.float32).T))
    bo2 = np.asarray(bo, np.float32).reshape(1, D)
    scale = np.float32(1.0 / np.sqrt(DK))
    n_pat = patterns.shape[0]
    mpat2 = np.ascontiguousarray(
        patterns.transpose(1, 0, 2).reshape(P, n_pat * SBLK))
    in_maps = []
    for c in range(N_CORES):
        cols = slice(c * EL, (c + 1) * EL)
        in_maps.append({
            "xT": xT,
            "wqT": _sbuf_tiled(np.asarray(Wq, np.float32).T[:, cols] * scale),
            "wkT": _sbuf_tiled(np.asarray(Wk, np.float32).T[:, cols]),
            "wvT": _sbuf_tiled(np.asarray(Wv, np.float32).T[:, cols]),
            "woT": woT,
            "bo": bo2,
            "mpat": mpat2,
        })
    return in_maps


def _run(inputs, trace=False):
    blocks, patterns = _classify_mask(inputs["mask"])
    nc = _build_program(blocks, patterns.shape[0])
    in_maps = _prepare_inputs(inputs["x"], inputs["Wq"], inputs["Wk"],
                              inputs["Wv"], inputs["Wo"], inputs["bo"],
                              patterns)
    res = run_bass_kernel_spmd(nc, in_maps, list(range(N_CORES)),
                               trace=trace)
    full = np.empty((B, S, D), np.float32)
    for c in range(N_CORES):
        b, j = divmod(c, NQ)
        full[b, j * SBLK:(j + 1) * SBLK, :] = res.results[c]["out"]
    return full, res


def kernel(**inputs) -> np.ndarray:
    out, _ = _run(inputs, trace=False)
    return out


# revision 6
# speedup vs baseline: 1.1846x; 1.0067x over previous
"""Multi-head causal self-attention on 8 Trainium2 NeuronCores.

Sharding: heads are split 2-per-core (tensor parallel); every core computes
QKV projections for its 2 heads over the full batch, runs attention, then two
half-size 8-core AllToAll collectives (one per head, the first overlapped
with the second head's attention sweep) redistribute the per-head outputs to
sequence-sharded form for the row-parallel output projection. Matmuls run in
float32r (TF32-like).

Reference semantics (torch nn.Linear convention, y = x @ W.T):
  Q = x @ Wq.T ; K = x @ Wk.T ; V = x @ Wv.T           (split into 16 heads)
  scores = Q K^T / sqrt(64), causal-masked, softmax
  out = (softmax(scores) @ V, concat heads) @ Wo.T + bo
"""

import sys
from contextlib import ExitStack

sys.path.insert(0, "/opt/trn_rl_repo")

import numpy as np

import concourse.bass as bass  # noqa: E402
import concourse.mybir as mybir  # noqa: E402
from concourse import bacc  # noqa: E402
from concourse.bass_utils import run_bass_kernel_spmd  # noqa: E402
from concourse.masks import make_identity  # noqa: E402
from concourse.tile import TileContext  # noqa: E402

B = 2
S = 2048
D = 1024
H = 16
DK = 64
N_CORES = 8
HPC = H // N_CORES          # heads per core = 2
EL = HPC * DK               # local embedding slice = 128
P = 128                     # partitions
SBLK = 512                  # q-block (free dim of score matmuls)
NQ = S // SBLK              # q-blocks per batch = 4
NKT = S // P                # k-tiles per batch = 16
ND = D // P                 # d-tiles = 8
BS = B * S                  # flattened (batch, seq) = 4096
F32 = mybir.dt.float32
F32R = mybir.dt.float32r


def _classify_mask(mask: np.ndarray):
    """Classify each (q-block, k-tile) block of the [S, S] mask.

    Returns (blocks, patterns):
      blocks[j] = list of (t, pat_idx or None) k-tiles with any valid entry
      patterns  = float32 [n_pat, P, SBLK] multiplicative masks in [k, q]
                  layout for partially-valid blocks (deduplicated).
    """
    mask = np.asarray(mask).astype(bool)
    patterns = []
    pat_index = {}
    blocks = []
    for j in range(NQ):
        row = []
        sub_q = mask[j * SBLK:(j + 1) * SBLK]
        for t in range(NKT):
            sub = sub_q[:, t * P:(t + 1) * P]
            if not sub.any():
                continue
            if sub.all():
                row.append((t, None))
                continue
            patT = np.ascontiguousarray(sub.T).astype(np.float32)  # [k, q]
            key = patT.tobytes()
            if key not in pat_index:
                pat_index[key] = len(patterns)
                patterns.append(patT)
            row.append((t, pat_index[key]))
        assert row, f"q-block {j} has no valid keys; unsupported mask"
        blocks.append(row)
    if not patterns:
        patterns.append(np.ones((P, SBLK), np.float32))
    return blocks, np.stack(patterns)


def _build_program(blocks, n_pat):
    nc = bacc.Bacc("TRN2", target_bir_lowering=False, debug=False,
                   num_devices=N_CORES)

    # ---- I/O ----------------------------------------------------------
    # xT: [D, B*S] (x transposed, batches concatenated along columns)
    # weight inputs are pre-tiled on the host into SBUF layout [P, ND*cols]
    xT = nc.declare_dram_parameter("xT", [D, BS], F32R, isOutput=False)
    wqT = nc.declare_dram_parameter("wqT", [P, ND * EL], F32R,
                                    isOutput=False)
    wkT = nc.declare_dram_parameter("wkT", [P, ND * EL], F32R,
                                    isOutput=False)
    wvT = nc.declare_dram_parameter("wvT", [P, ND * EL], F32R,
                                    isOutput=False)
    woT = nc.declare_dram_parameter("woT", [P, ND * D], F32R,
                                    isOutput=False)
    bo = nc.declare_dram_parameter("bo", [1, D], F32, isOutput=False)
    mpat = nc.declare_dram_parameter("mpat", [P, n_pat * SBLK], F32R,
                                     isOutput=False)
    out = nc.declare_dram_parameter("out", [SBLK, D], F32, isOutput=True)

    # collective bounce buffers (internal DRAM), one pair per head sweep
    a2a_in = [nc.dram_tensor(f"a2a_in{h}", [N_CORES, DK, SBLK], F32R)
              for h in range(HPC)]
    a2a_out = [nc.dram_tensor(f"a2a_out{h}", [N_CORES, DK, SBLK], F32R)
               for h in range(HPC)]

    with TileContext(nc) as tc, ExitStack() as ctx:
        const = ctx.enter_context(tc.tile_pool(name="const", bufs=1))
        persist = ctx.enter_context(tc.tile_pool(name="persist", bufs=1))

        # constants
        ident = const.tile([P, P], F32)
        make_identity(nc, ident)
        bo_sb = const.tile([1, D], F32)
        nc.sync.dma_start(out=bo_sb[:], in_=bo[:])
        bo_bc = const.tile([P, D], F32)
        nc.gpsimd.partition_broadcast(bo_bc[:], bo_sb[:])
        mpat_sb = const.tile([P, n_pat * SBLK], F32R, tag="mpat")
        nc.sync.dma_start(out=mpat_sb[:], in_=mpat[:])

        w_sb = {}
        for name, t in (("wq", wqT), ("wk", wkT), ("wv", wvT)):
            w = const.tile([P, ND * EL], F32R, name=f"w_{name}",
                           tag=f"w_{name}")
            nc.sync.dma_start(out=w[:], in_=t[:])
            w_sb[name] = w

        # persistent activations
        qT = persist.tile([P, BS], F32R, tag="qT")   # [EL, B*S]
        kT = persist.tile([P, BS], F32R, tag="kT")
        # V_aug: per k-tile [P, 2*65]; head h at cols [h*65, h*65+64],
        # ones at col h*65+64
        v_aug = [persist.tile([P, HPC * (DK + 1)], F32R,
                              name=f"vaug{i}", tag=f"vaug{i}")
                 for i in range(B * NKT)]
        stage = [persist.tile([P, SBLK], F32R, name=f"stg{i}",
                              tag=f"stg{i}")
                 for i in range(N_CORES)]
        woT_sb = persist.tile([P, ND * D], F32R, tag="woT")

        # ---- phase 1: projections -----------------------------------
        with ExitStack() as p1:
            xpool = p1.enter_context(tc.tile_pool(name="xT", bufs=ND + 1))
            vt_pool = p1.enter_context(tc.tile_pool(name="vt", bufs=1))
            ps_qk = p1.enter_context(
                tc.tile_pool(name="ps_qk", bufs=4, space="PSUM"))
            ps_tr = p1.enter_context(
                tc.tile_pool(name="ps_tr", bufs=2, space="PSUM"))

            HALF = BS // 2
            nsb = HALF // SBLK
            for half in range(2):
                xt = []
                for k in range(ND):
                    t = xpool.tile([P, HALF], F32R, tag="xt")
                    nc.sync.dma_start(
                        out=t[:],
                        in_=xT[k * P:(k + 1) * P,
                               half * HALF:(half + 1) * HALF])
                    xt.append(t)
                vt = vt_pool.tile([P, HALF], F32R, tag="vt")
                for name, dest in (("wq", qT), ("wk", kT), ("wv", vt)):
                    pss = [ps_qk.tile([P, SBLK], F32, tag="ps_qk",
                                      name=f"ps_{name}_{half}_{sb}")
                           for sb in range(nsb)]
                    for k in range(ND):
                        for sb in range(nsb):
                            nc.tensor.matmul(
                                pss[sb][:],
                                w_sb[name][:, bass.ts(k, EL)],
                                xt[k][:, bass.ts(sb, SBLK)],
                                start=(k == 0), stop=(k == ND - 1))
                    for sb in range(nsb):
                        if name == "wv":
                            nc.vector.tensor_copy(vt[:, bass.ts(sb, SBLK)],
                                                  pss[sb][:])
                        else:
                            osl = bass.ds(half * HALF + sb * SBLK, SBLK)
                            nc.vector.tensor_copy(dest[:, osl], pss[sb][:])
                # transpose VT -> V (per [P, P] block), augment ones col
                for i in range(HALF // P):
                    tile_idx = half * (HALF // P) + i
                    ps = ps_tr.tile([P, P], F32, tag="ps_tr")
                    nc.tensor.matmul(ps[:],
                                     vt[:, bass.ts(i, P)].bitcast(F32),
                                     ident[:], is_transpose=True)
                    va = v_aug[tile_idx]
                    va3 = va[:].rearrange("p (h e) -> p h e", h=HPC)
                    nc.vector.tensor_copy(
                        va3[:, :, 0:DK],
                        ps[:].rearrange("p (h e) -> p h e", h=HPC))
                    nc.vector.memset(va3[:, :, DK:DK + 1].bitcast(F32), 1.0)

        # WoT load (needed only in phase 3; issued here so the DMA
        # overlaps the attention phase)
        nc.sync.dma_start(out=woT_sb[:], in_=woT[:])

        # ---- phase 2: attention + per-head AllToAll -----------------
        with ExitStack() as p2:
            probs_pool = p2.enter_context(tc.tile_pool(name="probs",
                                                       bufs=5))
            small = p2.enter_context(tc.tile_pool(name="small", bufs=4))
            ps_sc = p2.enter_context(
                tc.tile_pool(name="ps_sc", bufs=3, space="PSUM"))
            ps_out = p2.enter_context(
                tc.tile_pool(name="ps_out", bufs=2, space="PSUM"))

            mpat3 = mpat_sb[:].rearrange("p (n q) -> p n q", n=n_pat)
            for h in range(HPC):
                hsl = bass.ds(h * DK, DK)
                for b in range(B):
                    for j in range(NQ):
                        q0 = b * S + j * SBLK
                        tiles = blocks[j]
                        po = ps_out.tile([P, SBLK], F32, tag="ps_out")
                        n_mm = 0
                        for c0 in range(0, len(tiles), 2):
                            pair = tiles[c0:c0 + 2]
                            w = len(pair) * SBLK
                            ps = ps_sc.tile([P, 2 * SBLK], F32, tag="ps_sc")
                            for i, (t, _pat) in enumerate(pair):
                                nc.tensor.matmul(
                                    ps[:, bass.ts(i, SBLK)],
                                    kT[hsl, bass.ds(b * S + t * P, P)],
                                    qT[hsl, bass.ds(q0, SBLK)],
                                    start=True, stop=True)
                            pr = probs_pool.tile([P, 2 * SBLK], F32R,
                                                 tag="probs")
                            nc.scalar.activation(
                                pr[:, 0:w], ps[:, 0:w],
                                mybir.ActivationFunctionType.Exp)
                            for i, (t, pat) in enumerate(pair):
                                if pat is not None:
                                    nc.vector.tensor_mul(
                                        pr[:, bass.ts(i, SBLK)],
                                        pr[:, bass.ts(i, SBLK)],
                                        mpat3[:, pat])
                            for i, (t, _pat) in enumerate(pair):
                                n_mm += 1
                                nc.tensor.matmul(
                                    po[0:DK + 1, :],
                                    v_aug[b * NKT + t][
                                        :, bass.ds(h * (DK + 1), DK + 1)],
                                    pr[:, bass.ts(i, SBLK)],
                                    start=(n_mm == 1),
                                    stop=(n_mm == len(tiles)))
                        sumb = small.tile([DK, SBLK], F32, tag="sumb")
                        nc.vector.tensor_copy(sumb[0:1, :],
                                              po[DK:DK + 1, :])
                        rb = small.tile([DK, SBLK], F32, tag="rb")
                        nc.gpsimd.partition_broadcast(rb[:], sumb[0:1, :])
                        nc.vector.reciprocal_approx_fast(out=rb[:],
                                                         in_=rb[:])
                        nc.vector.tensor_mul(
                            stage[b * NQ + j][h * DK:(h + 1) * DK, :],
                            po[0:DK, :], rb[:])
                        nc.sync.dma_start(
                            out=a2a_in[h][b * NQ + j],
                            in_=stage[b * NQ + j][h * DK:(h + 1) * DK, :])
                nc.gpsimd.collective_compute(
                    "AllToAll", mybir.AluOpType.bypass,
                    replica_groups=[list(range(N_CORES))],
                    ins=[a2a_in[h][:]], outs=[a2a_out[h][:]])

        # ---- phase 3: output projection -----------------------------
        with ExitStack() as p3:
            cat_pool = p3.enter_context(tc.tile_pool(name="cat", bufs=1))
            osb_pool = p3.enter_context(tc.tile_pool(name="osb", bufs=3))
            ps_f = p3.enter_context(
                tc.tile_pool(name="ps_f", bufs=3, space="PSUM"))

            cat = []
            for i in range(N_CORES):
                t = cat_pool.tile([P, SBLK], F32R, name=f"cat{i}",
                                  tag=f"cat{i}")
                for h in range(HPC):
                    nc.sync.dma_start(out=t[h * DK:(h + 1) * DK, :],
                                      in_=a2a_out[h][i])
                cat.append(t)
            for st in range(SBLK // P):
                pss = [ps_f.tile([P, SBLK], F32, tag="ps_f",
                                 name=f"ps_f_{st}_{eb}")
                       for eb in range(D // SBLK)]
                for k in range(ND):
                    for eb in range(D // SBLK):
                        nc.tensor.matmul(
                            pss[eb][:], cat[k][:, bass.ts(st, P)],
                            woT_sb[:, bass.ds(k * D + eb * SBLK, SBLK)],
                            start=(k == 0), stop=(k == ND - 1))
                for eb in range(D // SBLK):
                    ot = osb_pool.tile([P, SBLK], F32, tag="osb")
                    nc.vector.tensor_add(ot[:], pss[eb][:],
                                         bo_bc[:, bass.ts(eb, SBLK)])
                    nc.sync.dma_start(
                        out=out[st * P:(st + 1) * P,
                                eb * SBLK:(eb + 1) * SBLK],
                        in_=ot[:])

    nc.compile()
    return nc


def _sbuf_tiled(wT):
    # [D, E] -> [P, ND*E]: row p holds d-tiles k at columns [k*E, (k+1)*E)
    dd, e = wT.shape
    return np.ascontiguousarray(
        wT.reshape(dd // P, P, e).transpose(1, 0, 2).reshape(P, -1))


def _prepare_inputs(x, Wq, Wk, Wv, Wo, bo, patterns):
    x = np.asarray(x, np.float32)
    xT = np.ascontiguousarray(
        np.concatenate([x[b].T for b in range(B)], axis=1))
    woT = _sbuf_tiled(np.ascontiguousarray(np.asarray(Wo, np.float32).T))
    bo2 = np.asarray(bo, np.float32).reshape(1, D)
    scale = np.float32(1.0 / np.sqrt(DK))
    n_pat = patterns.shape[0]
    mpat2 = np.ascontiguousarray(
        patterns.transpose(1, 0, 2).reshape(P, n_pat * SBLK))
    in_maps = []
    for c in range(N_CORES):
        cols = slice(c * EL, (c + 1) * EL)
        in_maps.append({
            "xT": xT,
            "wqT": _sbuf_tiled(np.asarray(Wq, np.float32).T[:, cols] * scale),
            "wkT": _sbuf_tiled(np.asarray(Wk, np.float32).T[:, cols]),
            "wvT": _sbuf_tiled(np.asarray(Wv, np.float32).T[:, cols]),
            "woT": woT,
            "bo": bo2,
            "mpat": mpat2,
        })
    return in_maps


def _run(inputs, trace=False):
    blocks, patterns = _classify_mask(inputs["mask"])
    nc = _build_program(blocks, patterns.shape[0])
    in_maps = _prepare_inputs(inputs["x"], inputs["Wq"], inputs["Wk"],
                              inputs["Wv"], inputs["Wo"], inputs["bo"],
                              patterns)
    res = run_bass_kernel_spmd(nc, in_maps, list(range(N_CORES)),
                               trace=trace)
    full = np.empty((B, S, D), np.float32)
    for c in range(N_CORES):
        b, j = divmod(c, NQ)
        full[b, j * SBLK:(j + 1) * SBLK, :] = res.results[c]["out"]
    return full, res


def kernel(**inputs) -> np.ndarray:
    out, _ = _run(inputs, trace=False)
    return out


# revision 7
# speedup vs baseline: 1.2408x; 1.0475x over previous
"""Multi-head causal self-attention on 8 Trainium2 NeuronCores.

Sharding: heads are split 2-per-core (tensor parallel); every core computes
QKV projections for its 2 heads over the full batch, runs attention, then two
half-size 8-core AllToAll collectives (one per head, the first overlapped
with the second head's attention sweep) redistribute the per-head outputs to
sequence-sharded form for the row-parallel output projection. Matmuls run in
float32r (TF32-like).

Reference semantics (torch nn.Linear convention, y = x @ W.T):
  Q = x @ Wq.T ; K = x @ Wk.T ; V = x @ Wv.T           (split into 16 heads)
  scores = Q K^T / sqrt(64), causal-masked, softmax
  out = (softmax(scores) @ V, concat heads) @ Wo.T + bo
"""

import sys
from contextlib import ExitStack

sys.path.insert(0, "/opt/trn_rl_repo")

import numpy as np

import concourse.bass as bass  # noqa: E402
import concourse.mybir as mybir  # noqa: E402
from concourse import bacc  # noqa: E402
from concourse.bass_utils import run_bass_kernel_spmd  # noqa: E402
from concourse.masks import make_identity  # noqa: E402
from concourse.tile import TileContext  # noqa: E402

B = 2
S = 2048
D = 1024
H = 16
DK = 64
N_CORES = 8
HPC = H // N_CORES          # heads per core = 2
EL = HPC * DK               # local embedding slice = 128
P = 128                     # partitions
SBLK = 512                  # q-block (free dim of score matmuls)
NQ = S // SBLK              # q-blocks per batch = 4
NKT = S // P                # k-tiles per batch = 16
ND = D // P                 # d-tiles = 8
BS = B * S                  # flattened (batch, seq) = 4096
F32 = mybir.dt.float32
F32R = mybir.dt.float32r


def _classify_mask(mask: np.ndarray):
    """Classify each (q-block, k-tile) block of the [S, S] mask.

    Returns (blocks, patterns):
      blocks[j] = list of (t, pat_idx or None) k-tiles with any valid entry
      patterns  = float32 [n_pat, P, SBLK] multiplicative masks in [k, q]
                  layout for partially-valid blocks (deduplicated).
    """
    mask = np.asarray(mask).astype(bool)
    patterns = []
    pat_index = {}
    blocks = []
    for j in range(NQ):
        row = []
        sub_q = mask[j * SBLK:(j + 1) * SBLK]
        for t in range(NKT):
            sub = sub_q[:, t * P:(t + 1) * P]
            if not sub.any():
                continue
            if sub.all():
                row.append((t, None))
                continue
            patT = np.ascontiguousarray(sub.T).astype(np.float32)  # [k, q]
            key = patT.tobytes()
            if key not in pat_index:
                pat_index[key] = len(patterns)
                patterns.append(patT)
            row.append((t, pat_index[key]))
        assert row, f"q-block {j} has no valid keys; unsupported mask"
        blocks.append(row)
    if not patterns:
        patterns.append(np.ones((P, SBLK), np.float32))
    return blocks, np.stack(patterns)


def _build_program(blocks, n_pat):
    nc = bacc.Bacc("TRN2", target_bir_lowering=False, debug=False,
                   num_devices=N_CORES)

    # ---- I/O ----------------------------------------------------------
    # xT: [D, B*S] (x transposed, batches concatenated along columns)
    # weight inputs are pre-tiled on the host into SBUF layout [P, ND*cols]
    xT = nc.declare_dram_parameter("xT", [D, BS], F32R, isOutput=False)
    wqT = nc.declare_dram_parameter("wqT", [P, ND * EL], F32R,
                                    isOutput=False)
    wkT = nc.declare_dram_parameter("wkT", [P, ND * EL], F32R,
                                    isOutput=False)
    wvT = nc.declare_dram_parameter("wvT", [P, ND * EL], F32R,
                                    isOutput=False)
    woT = nc.declare_dram_parameter("woT", [P, ND * D], F32R,
                                    isOutput=False)
    bo = nc.declare_dram_parameter("bo", [1, D], F32, isOutput=False)
    mpat = nc.declare_dram_parameter("mpat", [P, n_pat * SBLK], F32R,
                                     isOutput=False)
    out = nc.declare_dram_parameter("out", [SBLK, D], F32, isOutput=True)

    # collective bounce buffers (internal DRAM), one pair per head sweep
    a2a_in = [nc.dram_tensor(f"a2a_in{h}", [N_CORES, DK, SBLK], F32R)
              for h in range(HPC)]
    a2a_out = [nc.dram_tensor(f"a2a_out{h}", [N_CORES, DK, SBLK], F32R)
               for h in range(HPC)]

    with TileContext(nc) as tc, ExitStack() as ctx:
        const = ctx.enter_context(tc.tile_pool(name="const", bufs=1))
        persist = ctx.enter_context(tc.tile_pool(name="persist", bufs=1))

        # constants
        ident = const.tile([P, P], F32)
        make_identity(nc, ident)
        bo_sb = const.tile([1, D], F32)
        nc.sync.dma_start(out=bo_sb[:], in_=bo[:])
        bo_bc = const.tile([P, D], F32)
        nc.gpsimd.partition_broadcast(bo_bc[:], bo_sb[:])
        w_sb = {}
        for name, t in (("wq", wqT), ("wk", wkT), ("wv", wvT)):
            w = const.tile([P, ND * EL], F32R, name=f"w_{name}",
                           tag=f"w_{name}")
            nc.sync.dma_start(out=w[:], in_=t[:])
            w_sb[name] = w
        mpat_sb = const.tile([P, n_pat * SBLK], F32R, tag="mpat")
        nc.sync.dma_start(out=mpat_sb[:], in_=mpat[:])

        # persistent activations
        qT = persist.tile([P, BS], F32R, tag="qT")   # [EL, B*S]
        kT = persist.tile([P, BS], F32R, tag="kT")
        # V_aug: per k-tile [P, 2*65]; head h at cols [h*65, h*65+64],
        # ones at col h*65+64
        v_aug = [persist.tile([P, HPC * (DK + 1)], F32R,
                              name=f"vaug{i}", tag=f"vaug{i}")
                 for i in range(B * NKT)]
        stage = [persist.tile([P, SBLK], F32R, name=f"stg{i}",
                              tag=f"stg{i}")
                 for i in range(N_CORES)]
        woT_sb = persist.tile([P, ND * D], F32R, tag="woT")

        # ---- phase 1: projections -----------------------------------
        with ExitStack() as p1:
            xpool = p1.enter_context(tc.tile_pool(name="xT", bufs=ND + 1))
            vt_pool = p1.enter_context(tc.tile_pool(name="vt", bufs=1))
            ps_qk = p1.enter_context(
                tc.tile_pool(name="ps_qk", bufs=6, space="PSUM"))
            ps_tr = p1.enter_context(
                tc.tile_pool(name="ps_tr", bufs=2, space="PSUM"))

            HALF = BS // 2
            nsb = HALF // SBLK
            for half in range(2):
                xt = []
                for k in range(ND):
                    t = xpool.tile([P, HALF], F32R, tag="xt")
                    nc.sync.dma_start(
                        out=t[:],
                        in_=xT[k * P:(k + 1) * P,
                               half * HALF:(half + 1) * HALF])
                    xt.append(t)
                vt = vt_pool.tile([P, HALF], F32R, tag="vt")
                for name, dest in (("wq", qT), ("wk", kT), ("wv", vt)):
                    pss = [ps_qk.tile([P, SBLK], F32, tag="ps_qk",
                                      name=f"ps_{name}_{half}_{sb}")
                           for sb in range(nsb)]
                    for k in range(ND):
                        for sb in range(nsb):
                            nc.tensor.matmul(
                                pss[sb][:],
                                w_sb[name][:, bass.ts(k, EL)],
                                xt[k][:, bass.ts(sb, SBLK)],
                                start=(k == 0), stop=(k == ND - 1))
                    for sb in range(nsb):
                        if name == "wv":
                            nc.vector.tensor_copy(vt[:, bass.ts(sb, SBLK)],
                                                  pss[sb][:])
                        else:
                            osl = bass.ds(half * HALF + sb * SBLK, SBLK)
                            nc.vector.tensor_copy(dest[:, osl], pss[sb][:])
                # transpose VT -> V (per [P, P] block), augment ones col
                for i in range(HALF // P):
                    tile_idx = half * (HALF // P) + i
                    ps = ps_tr.tile([P, P], F32, tag="ps_tr")
                    nc.tensor.matmul(ps[:],
                                     vt[:, bass.ts(i, P)].bitcast(F32),
                                     ident[:], is_transpose=True)
                    va = v_aug[tile_idx]
                    va3 = va[:].rearrange("p (h e) -> p h e", h=HPC)
                    nc.vector.tensor_copy(
                        va3[:, :, 0:DK],
                        ps[:].rearrange("p (h e) -> p h e", h=HPC))
                    nc.vector.memset(va3[:, :, DK:DK + 1].bitcast(F32), 1.0)

        # WoT load (needed only in phase 3; issued here so the DMA
        # overlaps the attention phase)
        nc.sync.dma_start(out=woT_sb[:], in_=woT[:])

        # ---- phase 2: attention + per-head AllToAll -----------------
        with ExitStack() as p2:
            probs_pool = p2.enter_context(tc.tile_pool(name="probs",
                                                       bufs=5))
            small = p2.enter_context(tc.tile_pool(name="small", bufs=4))
            ps_sc = p2.enter_context(
                tc.tile_pool(name="ps_sc", bufs=3, space="PSUM"))
            ps_out = p2.enter_context(
                tc.tile_pool(name="ps_out", bufs=2, space="PSUM"))

            mpat3 = mpat_sb[:].rearrange("p (n q) -> p n q", n=n_pat)
            for h in range(HPC):
                hsl = bass.ds(h * DK, DK)
                for b in range(B):
                    for j in range(NQ):
                        q0 = b * S + j * SBLK
                        tiles = blocks[j]
                        po = ps_out.tile([P, SBLK], F32, tag="ps_out")
                        pairs = [tiles[c0:c0 + 2]
                                 for c0 in range(0, len(tiles), 2)]
                        LAG = 2
                        n_mm = 0
                        pend = []
                        for idx in range(len(pairs) + LAG):
                            if idx < len(pairs):
                                pair = pairs[idx]
                                w = len(pair) * SBLK
                                ps = ps_sc.tile([P, 2 * SBLK], F32,
                                                tag="ps_sc")
                                for i, (t, _pat) in enumerate(pair):
                                    nc.tensor.matmul(
                                        ps[:, bass.ts(i, SBLK)],
                                        kT[hsl, bass.ds(b * S + t * P, P)],
                                        qT[hsl, bass.ds(q0, SBLK)],
                                        start=True, stop=True)
                                pr = probs_pool.tile([P, 2 * SBLK], F32R,
                                                     tag="probs")
                                nc.scalar.activation(
                                    pr[:, 0:w], ps[:, 0:w],
                                    mybir.ActivationFunctionType.Exp)
                                for i, (t, pat) in enumerate(pair):
                                    if pat is not None:
                                        nc.vector.tensor_mul(
                                            pr[:, bass.ts(i, SBLK)],
                                            pr[:, bass.ts(i, SBLK)],
                                            mpat3[:, pat])
                                pend.append((pair, pr))
                            if idx >= LAG:
                                pair, pr = pend[idx - LAG]
                                for i, (t, _pat) in enumerate(pair):
                                    n_mm += 1
                                    nc.tensor.matmul(
                                        po[0:DK + 1, :],
                                        v_aug[b * NKT + t][
                                            :, bass.ds(h * (DK + 1),
                                                       DK + 1)],
                                        pr[:, bass.ts(i, SBLK)],
                                        start=(n_mm == 1),
                                        stop=(n_mm == len(tiles)))
                        sumb = small.tile([DK, SBLK], F32, tag="sumb")
                        nc.vector.tensor_copy(sumb[0:1, :],
                                              po[DK:DK + 1, :])
                        rb = small.tile([DK, SBLK], F32, tag="rb")
                        nc.gpsimd.partition_broadcast(rb[:], sumb[0:1, :])
                        nc.vector.reciprocal_approx_fast(out=rb[:],
                                                         in_=rb[:])
                        nc.vector.tensor_mul(
                            stage[b * NQ + j][h * DK:(h + 1) * DK, :],
                            po[0:DK, :], rb[:])
                        nc.sync.dma_start(
                            out=a2a_in[h][b * NQ + j],
                            in_=stage[b * NQ + j][h * DK:(h + 1) * DK, :])
                nc.gpsimd.collective_compute(
                    "AllToAll", mybir.AluOpType.bypass,
                    replica_groups=[list(range(N_CORES))],
                    ins=[a2a_in[h][:]], outs=[a2a_out[h][:]])

        # ---- phase 3: output projection -----------------------------
        with ExitStack() as p3:
            cat_pool = p3.enter_context(tc.tile_pool(name="cat", bufs=1))
            osb_pool = p3.enter_context(tc.tile_pool(name="osb", bufs=3))
            ps_f = p3.enter_context(
                tc.tile_pool(name="ps_f", bufs=3, space="PSUM"))

            cat = []
            for i in range(N_CORES):
                t = cat_pool.tile([P, SBLK], F32R, name=f"cat{i}",
                                  tag=f"cat{i}")
                for h in range(HPC):
                    nc.sync.dma_start(out=t[h * DK:(h + 1) * DK, :],
                                      in_=a2a_out[h][i])
                cat.append(t)
            for st in range(SBLK // P):
                pss = [ps_f.tile([P, SBLK], F32, tag="ps_f",
                                 name=f"ps_f_{st}_{eb}")
                       for eb in range(D // SBLK)]
                for k in range(ND):
                    for eb in range(D // SBLK):
                        nc.tensor.matmul(
                            pss[eb][:], cat[k][:, bass.ts(st, P)],
                            woT_sb[:, bass.ds(k * D + eb * SBLK, SBLK)],
                            start=(k == 0), stop=(k == ND - 1))
                for eb in range(D // SBLK):
                    ot = osb_pool.tile([P, SBLK], F32, tag="osb")
                    nc.vector.tensor_add(ot[:], pss[eb][:],
                                         bo_bc[:, bass.ts(eb, SBLK)])
                    nc.sync.dma_start(
                        out=out[st * P:(st + 1) * P,
                                eb * SBLK:(eb + 1) * SBLK],
                        in_=ot[:])

    nc.compile()
    return nc


def _sbuf_tiled(wT):
    # [D, E] -> [P, ND*E]: row p holds d-tiles k at columns [k*E, (k+1)*E)
    dd, e = wT.shape
    return np.ascontiguousarray(
        wT.reshape(dd // P, P, e).transpose(1, 0, 2).reshape(P, -1))


def _prepare_inputs(x, Wq, Wk, Wv, Wo, bo, patterns):
    x = np.asarray(x, np.float32)
    xT = np.ascontiguousarray(
        np.concatenate([x[b].T for b in range(B)], axis=1))
    woT = _sbuf_tiled(np.ascontiguousarray(np.asarray(Wo, np.float32).T))
    bo2 = np.asarray(bo, np.float32).reshape(1, D)
    scale = np.float32(1.0 / np.sqrt(DK))
    n_pat = patterns.shape[0]
    mpat2 = np.ascontiguousarray(
        patterns.transpose(1, 0, 2).reshape(P, n_pat * SBLK))
    in_maps = []
    for c in range(N_CORES):
        cols = slice(c * EL, (c + 1) * EL)
        in_maps.append({
            "xT": xT,
            "wqT": _sbuf_tiled(np.asarray(Wq, np.float32).T[:, cols] * scale),
            "wkT": _sbuf_tiled(np.asarray(Wk, np.float32).T[:, cols]),
            "wvT": _sbuf_tiled(np.asarray(Wv, np.float32).T[:, cols]),
            "woT": woT,
            "bo": bo2,
            "mpat": mpat2,
        })
    return in_maps


def _run(inputs, trace=False):
    blocks, patterns = _classify_mask(inputs["mask"])
    nc = _build_program(blocks, patterns.shape[0])
    in_maps = _prepare_inputs(inputs["x"], inputs["Wq"], inputs["Wk"],
                              inputs["Wv"], inputs["Wo"], inputs["bo"],
                              patterns)
    res = run_bass_kernel_spmd(nc, in_maps, list(range(N_CORES)),
                               trace=trace)
    full = np.empty((B, S, D), np.float32)
    for c in range(N_CORES):
        b, j = divmod(c, NQ)
        full[b, j * SBLK:(j + 1) * SBLK, :] = res.results[c]["out"]
    return full, res


def kernel(**inputs) -> np.ndarray:
    out, _ = _run(inputs, trace=False)
    return out
